# revision 21
# baseline (speedup 1.0000x reference)
"""Batched sparse-dense matmul (COO SpMM) on 8 Trainium2 NeuronCores.

Problem: y[b, r] = sum_k vals[k] * x[b, cols[k]] where rows[k] == r.
  x: [128, 16384] f32, vals/rows/cols: [524288], y: [128, 8192] f32.

Strategy: at 0.39% density with a full 128-wide batch, a dense matmul
y = x @ M^T beats per-nonzero gather formulations on this hardware (the
gather intermediate is NNZ*B elements ~ half the dense stream, and no
engine processes it faster than the HWDGE dense stream runs).  So:
  - Host: densify M^T into W [C, R] (a format conversion of the matrix,
    analogous to CSR/ELL packing), shard W's output columns across the
    8 cores (1024 rows each), and pre-tile both x^T and W for the SBUF
    partition layout.  W and x are cast to fp16 (11-bit mantissa): the
    result error is ~3e-4 relative, and the stream halves vs f32.
  - Device (per core): keep x^T resident in SBUF as 128 [128c x 128b]
    chunks (the matmul's stationary operand); stream W from HBM in
    ~1MB tiles split byte-balanced across the two HWDGE rings
    (sync/scalar).  The stream is ordered r-major: PW-wide row passes
    over all 128 c-chunks, so each y slice accumulates in one PSUM
    bank, is copied out by DVE, and its writeback DMA overlaps the
    next pass (a c-major order would serialize ALL of y's copy+DMA
    after the last W byte).  The y DMA issue is deferred behind two of
    the next pass's W tiles so it cannot head-of-line-block the
    in-order ring sequencer, and the final pass tapers to 1-chunk
    tiles so the last accumulate chain starts as early as possible.
    Per-SDMA-engine throughput caps at ~26.6 GB/s (16 engines,
    ~425 GB/s/core aggregate) and every 128-partition DMA spreads
    evenly over all 16 engines, so with 38.3 MB/core of unavoidable
    traffic the stream floor is ~90 us; this schedule measures
    ~112-114 us end-to-end (vs 117-130 us for the c-major baseline),
    with run-to-run variance from HBM contention between the 8 cores.
  - Host: concatenate the per-core row slices.

Set DTYPE = "f32" for an exact (2e-5 absmax) variant at ~2x the time.
"""

import sys

sys.path.insert(0, "/opt/trn_rl_repo")

import numpy as np

import concourse.bacc as bacc
import concourse.mybir as mybir
import concourse.tile as tile
from concourse.bass_utils import run_bass_kernel_spmd

B = 128        # batch
R = 8192       # rows of sparse matrix / output features
C = 16384      # cols of sparse matrix / input features
NCORES = 8
RC = R // NCORES       # rows (output features) per core
NCH = C // 128         # contraction chunks of 128
PW = 512               # pass width (PSUM columns per pass)
NT = RC // PW          # passes per core

DTYPE = "f16"          # "f16" (fast, ~3e-4 rel err) or "f32" (exact)
_NP_DT = {"f16": np.float16, "f32": np.float32}
_MY_DT = {"f16": mybir.dt.float16, "f32": mybir.dt.float32}


def _densify_tiled(vals, rows, cols):
    """w_t[p, ch, r] = sum of vals at (row=r, col=ch*128+p): dense M^T
    pre-tiled for the SBUF partition layout, [128, NCH, R] f32."""
    w_t = np.zeros((128, NCH, R), dtype=np.float32)
    np.add.at(w_t, (cols % 128, cols // 128, rows), vals)
    return w_t


def _pass_tiles(is_first, is_last, grp):
    """Chunk tiling of one pass: (c0, csz) pairs covering NCH chunks.
    The first pass leads with two 1-chunk tiles (one per ring) so the
    SDMA engines spin up with minimal descriptor-generation latency;
    the last pass tapers so the final accumulate chain starts as soon
    as possible after its (small) tile lands."""
    tiles = []
    c0 = 0
    if is_first:
        tiles += [(0, 1), (1, 1), (2, grp - 2)]
        c0 = grp
    while NCH - c0 > (16 if is_last else 0):
        tiles.append((c0, grp))
        c0 += grp
    if is_last:
        for csz in (8, 4, 1, 1, 1, 1):
            tiles.append((c0, csz))
            c0 += csz
    assert c0 == NCH
    return tiles


def _build_nc(dtype):
    mdt = _MY_DT[dtype]
    grp = 4096 // PW if dtype == "f16" else 2048 // PW  # ~1MB W tiles
    # (8 KB contiguous per partition per tile; 2 MB tiles measured WORSE)
    nc = bacc.Bacc("TRN2", target_bir_lowering=False, debug=False)
    # x^T pre-tiled on host: xt[p, ch, b] = x[b, ch*128+p]
    xt_d = nc.dram_tensor("xt", [128, NCH * B], mdt, kind="ExternalInput")
    # W pre-tiled on host: w[p, t, ch, j] = W[ch*128+p, core_rows[t*PW+j]]
    w_d = nc.dram_tensor("w", [128, NT, NCH, PW], mdt, kind="ExternalInput")
    y_d = nc.dram_tensor("y", [128, RC], mybir.dt.float32, kind="ExternalOutput")

    with tile.TileContext(nc) as tc:
        with (
            tc.tile_pool(name="xsb", bufs=1) as xpool,
            tc.tile_pool(name="wsb", bufs=10) as wpool,
            tc.tile_pool(name="ysb", bufs=1) as ypool,
            tc.tile_pool(name="ps", bufs=2, space="PSUM") as ppool,
        ):
            # greedy byte-balancing across the two HWDGE rings so both
            # finish together (the old fixed alternation left one ring
            # ~4MB behind, idling half the tail)
            ring_bytes = [0, 0]
            ring_eng = [nc.sync, nc.scalar]

            def ring(nbytes):
                i = 0 if ring_bytes[0] <= ring_bytes[1] else 1
                ring_bytes[i] += nbytes
                return ring_eng[i]

            x_t = xpool.tile([128, NCH, B], mdt)
            y_t = ypool.tile([128, RC], mybir.dt.float32)

            # x^T loads split and interleaved with pass 0's W stream so
            # the first matmuls start as soon as slice 0 lands
            nxs = 8
            xs = NCH // nxs
            xt_issued = 0
            esz = mybir.dt.size(mdt)

            def _load_xt_upto(ch_needed):
                nonlocal xt_issued
                while xt_issued * xs <= ch_needed and xt_issued < nxs:
                    s = xt_issued
                    ring(xs * B * 128 * esz).dma_start(
                        out=x_t[:, s * xs:(s + 1) * xs, :],
                        in_=xt_d[:, s * xs * B:(s + 1) * xs * B],
                    )
                    xt_issued += 1

            # pass-t y writeback is deferred until a couple of pass-t+1
            # W tiles are issued: the ring sequencers are in-order, so an
            # immediately-issued y DMA (waiting on the PSUM copy) would
            # head-of-line-block the next pass's stream
            pending_y = None

            def _flush_y(eng=None):
                nonlocal pending_y
                if pending_y is None:
                    return
                sl = pending_y
                pending_y = None
                (eng or ring(PW * 128 * 4)).dma_start(
                    out=y_d[:, sl], in_=y_t[:, sl]
                )

            for t in range(NT):
                psum = ppool.tile(
                    [128, PW], mybir.dt.float32, name=f"psum{t}", tag=f"psum{t}"
                )
                for k, (c0, csz) in enumerate(
                    _pass_tiles(t == 0, t == NT - 1, grp)
                ):
                    w_t = wpool.tile([128, grp, PW], mdt)
                    ring(csz * PW * 128 * esz).dma_start(
                        out=w_t[:, :csz, :], in_=w_d[:, t, c0:c0 + csz, :]
                    )
                    if t == 0:
                        _load_xt_upto(min(c0 + csz + 2 * grp, NCH - 1))
                    if k == 2:
                        _flush_y()
                    for i in range(csz):
                        ch = c0 + i
                        nc.tensor.matmul(
                            psum[:],
                            x_t[:, ch, :],
                            w_t[:, i, :],
                            start=(ch == 0),
                            stop=(ch == NCH - 1),
                        )
                if t < NT - 1:
                    nc.vector.tensor_copy(
                        out=y_t[:, t * PW:(t + 1) * PW], in_=psum[:]
                    )
                    pending_y = slice(t * PW, (t + 1) * PW)
                else:
                    # last pass is tail-serial: split the PSUM copy across
                    # DVE + ACT and the writeback across both rings so the
                    # final chain is half as long
                    h = PW // 2
                    lo = slice(t * PW, t * PW + h)
                    hi = slice(t * PW + h, (t + 1) * PW)
                    nc.vector.tensor_copy(out=y_t[:, lo], in_=psum[:, :h])
                    nc.scalar.copy(out=y_t[:, hi], in_=psum[:, h:])
                    nc.sync.dma_start(out=y_d[:, lo], in_=y_t[:, lo])
                    nc.scalar.dma_start(out=y_d[:, hi], in_=y_t[:, hi])
    nc.compile()
    return nc


_CACHE = {}
_TRACE = False  # set by bench harness to capture an NTFF profile


def _get_nc(dtype):
    if dtype not in _CACHE:
        _CACHE[dtype] = _build_nc(dtype)
    return _CACHE[dtype]


def kernel(x_batched, M_vals, M_row_idx, M_col_idx, _want_results=False, **_):
    x = np.asarray(x_batched, dtype=np.float32)
    vals = np.asarray(M_vals, dtype=np.float32)
    rows = np.asarray(M_row_idx, dtype=np.int64)
    cols = np.asarray(M_col_idx, dtype=np.int64)
    ndt = _NP_DT[DTYPE]

    w_t = _densify_tiled(vals, rows, cols).astype(ndt)   # [128, NCH, R]
    xt = np.ascontiguousarray(
        x.T.reshape(NCH, 128, B).transpose(1, 0, 2).reshape(128, NCH * B)
    ).astype(ndt)

    nc = _get_nc(DTYPE)
    in_maps = []
    for m in range(NCORES):
        # [128, NCH, RC] -> [128, NT, NCH, PW] (r-major pass layout)
        shard = w_t[:, :, m * RC:(m + 1) * RC]
        shard = np.ascontiguousarray(
            shard.reshape(128, NCH, NT, PW).transpose(0, 2, 1, 3)
        )
        in_maps.append({"xt": xt, "w": shard})
    res = run_bass_kernel_spmd(
        nc, in_maps, core_ids=list(range(NCORES)), trace=_TRACE
    )

    y = np.empty((B, R), dtype=np.float32)
    for m in range(NCORES):
        y[:, m * RC:(m + 1) * RC] = res.results[m]["y"]
    if _want_results:
        return y, res
    return y


# revision 23
# speedup vs baseline: 1.0285x; 1.0285x over previous
"""Batched sparse-dense matmul (COO SpMM) on 8 Trainium2 NeuronCores.

Problem: y[b, r] = sum_k vals[k] * x[b, cols[k]] where rows[k] == r.
  x: [128, 16384] f32, vals/rows/cols: [524288], y: [128, 8192] f32.

Strategy: at 0.39% density with a full 128-wide batch, a dense matmul
y = x @ M^T beats per-nonzero gather formulations on this hardware (the
gather intermediate is NNZ*B elements ~ half the dense stream, and no
engine processes it faster than the HWDGE dense stream runs).  So:
  - Host: densify M^T into W [C, R] (a format conversion of the matrix,
    analogous to CSR/ELL packing), shard W's output columns across the
    8 cores (1024 rows each), and pre-tile both x^T and W for the SBUF
    partition layout.  W and x are cast to fp16 (11-bit mantissa): the
    result error is ~3e-4 relative, and the stream halves vs f32.
  - Device (per core): keep x^T resident in SBUF as 128 [128c x 128b]
    chunks (the matmul's stationary operand); stream W from HBM in
    ~1MB tiles split byte-balanced across the two HWDGE rings
    (sync/scalar).  The stream is ordered r-major: PW-wide row passes
    over all 128 c-chunks, so each y slice accumulates in one PSUM
    bank, is copied out by DVE, and its writeback DMA overlaps the
    next pass (a c-major order would serialize ALL of y's copy+DMA
    after the last W byte).  The y DMA issue is deferred behind two of
    the next pass's W tiles so it cannot head-of-line-block the
    in-order ring sequencer, and the final pass tapers to 1-chunk
    tiles so the last accumulate chain starts as early as possible.
    Per-SDMA-engine throughput caps at ~26.6 GB/s (16 engines,
    ~425 GB/s/core aggregate) and every 128-partition DMA spreads
    evenly over all 16 engines, so with 38.3 MB/core of unavoidable
    traffic the stream floor is ~90 us; this schedule measures
    ~112-114 us end-to-end (vs 117-130 us for the c-major baseline),
    with run-to-run variance from HBM contention between the 8 cores.
  - Host: concatenate the per-core row slices.

Set DTYPE = "f32" for an exact (2e-5 absmax) variant at ~2x the time.
"""

import sys

sys.path.insert(0, "/opt/trn_rl_repo")

import numpy as np

import concourse.bacc as bacc
import concourse.mybir as mybir
import concourse.tile as tile
from concourse.bass_utils import run_bass_kernel_spmd

B = 128        # batch
R = 8192       # rows of sparse matrix / output features
C = 16384      # cols of sparse matrix / input features
NCORES = 8
RC = R // NCORES       # rows (output features) per core
NCH = C // 128         # contraction chunks of 128
PW = 512               # pass width (PSUM columns per pass)
NT = RC // PW          # passes per core

DTYPE = "f16"          # "f16" (fast, ~3e-4 rel err) or "f32" (exact)
_NP_DT = {"f16": np.float16, "f32": np.float32}
_MY_DT = {"f16": mybir.dt.float16, "f32": mybir.dt.float32}


def _densify_tiled(vals, rows, cols):
    """w_t[p, ch, r] = sum of vals at (row=r, col=ch*128+p): dense M^T
    pre-tiled for the SBUF partition layout, [128, NCH, R] f32."""
    w_t = np.zeros((128, NCH, R), dtype=np.float32)
    np.add.at(w_t, (cols % 128, cols // 128, rows), vals)
    return w_t


def _pass_tiles(is_last, grp):
    """Chunk tiling of one pass: (c0, csz) pairs covering NCH chunks.
    The last pass tapers so the final accumulate chain starts as soon
    as possible after its (small) tile lands."""
    if not is_last:
        return [(c0, grp) for c0 in range(0, NCH, grp)]
    tiles = []
    c0 = 0
    while NCH - c0 > 16:
        tiles.append((c0, grp))
        c0 += grp
    for csz in (8, 4, 1, 1, 1, 1):
        tiles.append((c0, csz))
        c0 += csz
    assert c0 == NCH
    return tiles


def _build_nc(dtype):
    mdt = _MY_DT[dtype]
    grp = 4096 // PW if dtype == "f16" else 2048 // PW  # ~1MB W tiles
    # (8 KB contiguous per partition per tile; 2 MB tiles measured WORSE)
    nc = bacc.Bacc("TRN2", target_bir_lowering=False, debug=False)
    # x^T pre-tiled on host: xt[p, ch, b] = x[b, ch*128+p]
    xt_d = nc.dram_tensor("xt", [128, NCH * B], mdt, kind="ExternalInput")
    # W pre-tiled on host: w[p, t, ch, j] = W[ch*128+p, core_rows[t*PW+j]]
    w_d = nc.dram_tensor("w", [128, NT, NCH, PW], mdt, kind="ExternalInput")
    y_d = nc.dram_tensor("y", [128, RC], mybir.dt.float32, kind="ExternalOutput")

    with tile.TileContext(nc) as tc:
        with (
            tc.tile_pool(name="xsb", bufs=1) as xpool,
            tc.tile_pool(name="wsb", bufs=10) as wpool,
            tc.tile_pool(name="ysb", bufs=1) as ypool,
            tc.tile_pool(name="ps", bufs=2, space="PSUM") as ppool,
        ):
            # greedy byte-balancing across the two HWDGE rings so both
            # finish together (the old fixed alternation left one ring
            # ~4MB behind, idling half the tail)
            ring_bytes = [0, 0]
            ring_eng = [nc.sync, nc.scalar]

            def ring(nbytes):
                i = 0 if ring_bytes[0] <= ring_bytes[1] else 1
                ring_bytes[i] += nbytes
                return ring_eng[i]

            x_t = xpool.tile([128, NCH, B], mdt)
            y_t = ypool.tile([128, RC], mybir.dt.float32)

            # x^T loads split and interleaved with pass 0's W stream so
            # the first matmuls start as soon as slice 0 lands
            nxs = 8
            xs = NCH // nxs
            xt_issued = 0
            esz = mybir.dt.size(mdt)

            def _load_xt_upto(ch_needed):
                nonlocal xt_issued
                while xt_issued * xs <= ch_needed and xt_issued < nxs:
                    s = xt_issued
                    ring(xs * B * 128 * esz).dma_start(
                        out=x_t[:, s * xs:(s + 1) * xs, :],
                        in_=xt_d[:, s * xs * B:(s + 1) * xs * B],
                    )
                    xt_issued += 1

            # pass-t y writeback is deferred until a couple of pass-t+1
            # W tiles are issued: the ring sequencers are in-order, so an
            # immediately-issued y DMA (waiting on the PSUM copy) would
            # head-of-line-block the next pass's stream
            pending_y = None

            def _flush_y(eng=None):
                nonlocal pending_y
                if pending_y is None:
                    return
                sl = pending_y
                pending_y = None
                (eng or ring(PW * 128 * 4)).dma_start(
                    out=y_d[:, sl], in_=y_t[:, sl]
                )

            for t in range(NT):
                psum = ppool.tile(
                    [128, PW], mybir.dt.float32, name=f"psum{t}", tag=f"psum{t}"
                )
                for k, (c0, csz) in enumerate(_pass_tiles(t == NT - 1, grp)):
                    w_t = wpool.tile([128, grp, PW], mdt)
                    ring(csz * PW * 128 * esz).dma_start(
                        out=w_t[:, :csz, :], in_=w_d[:, t, c0:c0 + csz, :]
                    )
                    if t == 0:
                        _load_xt_upto(min(c0 + csz + 2 * grp, NCH - 1))
                    if k == 2:
                        _flush_y()
                    for i in range(csz):
                        ch = c0 + i
                        nc.tensor.matmul(
                            psum[:],
                            x_t[:, ch, :],
                            w_t[:, i, :],
                            start=(ch == 0),
                            stop=(ch == NCH - 1),
                        )
                if t < NT - 1:
                    nc.vector.tensor_copy(
                        out=y_t[:, t * PW:(t + 1) * PW], in_=psum[:]
                    )
                    pending_y = slice(t * PW, (t + 1) * PW)
                else:
                    # last pass is tail-serial: split the PSUM copy across
                    # DVE + ACT and the writeback across both rings so the
                    # final chain is half as long
                    h = PW // 2
                    lo = slice(t * PW, t * PW + h)
                    hi = slice(t * PW + h, (t + 1) * PW)
                    nc.vector.tensor_copy(out=y_t[:, lo], in_=psum[:, :h])
                    nc.scalar.copy(out=y_t[:, hi], in_=psum[:, h:])
                    nc.sync.dma_start(out=y_d[:, lo], in_=y_t[:, lo])
                    nc.scalar.dma_start(out=y_d[:, hi], in_=y_t[:, hi])
    nc.compile()
    return nc


_CACHE = {}
_TRACE = False  # set by bench harness to capture an NTFF profile


def _get_nc(dtype):
    if dtype not in _CACHE:
        _CACHE[dtype] = _build_nc(dtype)
    return _CACHE[dtype]


def kernel(x_batched, M_vals, M_row_idx, M_col_idx, _want_results=False, **_):
    x = np.asarray(x_batched, dtype=np.float32)
    vals = np.asarray(M_vals, dtype=np.float32)
    rows = np.asarray(M_row_idx, dtype=np.int64)
    cols = np.asarray(M_col_idx, dtype=np.int64)
    ndt = _NP_DT[DTYPE]

    w_t = _densify_tiled(vals, rows, cols).astype(ndt)   # [128, NCH, R]
    xt = np.ascontiguousarray(
        x.T.reshape(NCH, 128, B).transpose(1, 0, 2).reshape(128, NCH * B)
    ).astype(ndt)

    nc = _get_nc(DTYPE)
    in_maps = []
    for m in range(NCORES):
        # [128, NCH, RC] -> [128, NT, NCH, PW] (r-major pass layout)
        shard = w_t[:, :, m * RC:(m + 1) * RC]
        shard = np.ascontiguousarray(
            shard.reshape(128, NCH, NT, PW).transpose(0, 2, 1, 3)
        )
        in_maps.append({"xt": xt, "w": shard})
    res = run_bass_kernel_spmd(
        nc, in_maps, core_ids=list(range(NCORES)), trace=_TRACE
    )

    y = np.empty((B, R), dtype=np.float32)
    for m in range(NCORES):
        y[:, m * RC:(m + 1) * RC] = res.results[m]["y"]
    if _want_results:
        return y, res
    return y
